# revision 3
# baseline (speedup 1.0000x reference)
"""Trainium2 Bass kernel for nn_MoELayer_63556926046582.

8-core SPMD plan (everything on-device stays in *interleaved* token order,
row 2i = batch-0 token i, row 2i+1 = batch-1 token i, which makes the dual
interleaved attention an ordinary causal attention over 2048 tokens):

  - attention: tensor-parallel over heads (2 of 16 per core), QKV from
    transposed activations, flash-style causal attention in the S^T/P^T
    orientation (no transposes), softmax without max-subtraction.
  - AllGather of per-core o^T slices (bf16), redundant o-proj + rmsnorm.
  - router math on device; expert token lists host-planned from the index
    inputs; expert-parallel MoE (4 experts/core) via indirect-DMA gather,
    DMA-transpose, SwiGLU, indirect scatter-add with CCE accumulate.
  - shared expert tensor-parallel over hidden; one ReduceScatter combines.
"""
import sys

sys.path.insert(0, "/opt/trn_rl_repo")

import numpy as np
import ml_dtypes

import concourse.bass as bass
import concourse.bacc as bacc
import concourse.tile as tile
from concourse import mybir
from concourse.bass_utils import run_bass_kernel_spmd

BF = mybir.dt.bfloat16
F32 = mybir.dt.float32
I32 = mybir.dt.int32
AF = mybir.ActivationFunctionType
OP = mybir.AluOpType

P = 128
DIM = 1024
HEADS = 16
HEAD = 64
S = 1024
T2 = 2 * S
K = 4
FD = 512
DS = 2048
NC = 8
EPS = 1e-5
THETA = 10000.0
TROWS = T2 + P

KC = DIM // P
TC = T2 // P
NB = T2 // 512


def _bf(x):
    return np.ascontiguousarray(x.astype(ml_dtypes.bfloat16))


def _f32(x):
    return np.ascontiguousarray(x.astype(np.float32))


# --------------------------------------------------------------------------
# host-side preparation
# --------------------------------------------------------------------------

def _prepare(inputs):
    x_input = np.asarray(inputs["x_input"], np.float32)
    p_idx = np.asarray(inputs["p_indices"]).astype(np.int64)
    f_idx = np.asarray(inputs["f_indices"]).astype(np.int64)
    p_val = np.asarray(inputs["p_values"], np.float32)
    f_val = np.asarray(inputs["f_values"], np.float32)
    attn_nw = np.asarray(inputs["attn_norm_w"], np.float32)
    ffn_nw = np.asarray(inputs["ffn_norm_w"], np.float32)
    W_attn = np.asarray(inputs["W_attn"], np.float32)
    W_o = np.asarray(inputs["W_attn_o"], np.float32)
    ffn_up = np.asarray(inputs["ffn_up_w"], np.float32)
    ffn_dn = np.asarray(inputs["ffn_down_w"], np.float32)
    p_ffn = np.asarray(inputs["p_ffn_experts"], np.float32)
    f_ffn = np.asarray(inputs["f_ffn_experts"], np.float32)
    p_keys = np.asarray(inputs["p_token_keys"], np.float32)
    f_keys = np.asarray(inputs["f_token_keys"], np.float32)
    p_bias = np.asarray(inputs["p_router_bias"], np.float32)
    f_bias = np.asarray(inputs["f_router_bias"], np.float32)

    x_il = np.empty((T2, DIM), np.float32)
    x_il[0::2] = x_input[0]
    x_il[1::2] = x_input[1]

    inv = (1.0 / THETA) ** (np.arange(0, HEAD, 2, dtype=np.float32) / HEAD)
    pos = (np.arange(T2) // 2).astype(np.float32)
    fr = pos[:, None] * inv[None, :]
    cosT = np.cos(fr).T.astype(np.float32)
    sinT = np.sin(fr).T.astype(np.float32)
    cos128 = np.vstack([cosT, cosT, cosT, cosT])
    sin128s = np.vstack([sinT, -sinT, sinT, -sinT])

    masks = np.zeros((P, 4 * 512), np.float32)
    r = np.arange(P)[:, None]
    c = np.arange(512)[None, :]
    for j in range(4):
        masks[:, j * 512:(j + 1) * 512] = (c >= j * P + r)

    vals_adj = np.empty((T2, K), np.float32)
    vals_adj[0::2] = p_val + p_bias[p_idx]
    vals_adj[1::2] = f_val + f_bias[f_idx]
    sel = np.zeros((T2, K, 32), np.float32)
    t_even = np.arange(0, T2, 2)
    t_odd = np.arange(1, T2, 2)
    for k in range(K):
        sel[t_even, k, p_idx[:, k]] = 1.0
        sel[t_odd, k, 16 + f_idx[:, k]] = 1.0
    sel = sel.reshape(T2, K * 32)
    keys_cat = np.concatenate([p_keys, f_keys], 1) * ffn_nw[:, None]

    W_attn_n = W_attn * attn_nw[:, None]
    ffn_up_n = ffn_up * ffn_nw[:, None]

    per_core = []
    NT = 1
    core_slots = []
    for c0 in range(NC):
        slots = [("p", 2 * c0), ("p", 2 * c0 + 1), ("f", 2 * c0), ("f", 2 * c0 + 1)]
        core_slots.append(slots)
        for half, e in slots:
            idx = p_idx if half == "p" else f_idx
            n = int(np.unique(np.where((idx == e).any(1))[0]).size)
            NT = max(NT, (n + P - 1) // P)
    meta = {"NT": NT}

    for c0 in range(NC):
        slots = core_slots[c0]
        wg = np.empty((4 * DIM, FD), np.float32)
        wu = np.empty((4 * DIM, FD), np.float32)
        wdT = np.empty((4 * FD, DIM), np.float32)
        gidx = np.full((P, 4 * NT), T2, np.int32)
        selE4 = np.zeros((T2, K, 4), np.float32)
        for i, (half, e) in enumerate(slots):
            blk = p_ffn if half == "p" else f_ffn
            wg[i * DIM:(i + 1) * DIM] = blk[0, e] * ffn_nw[:, None]
            wu[i * DIM:(i + 1) * DIM] = blk[1, e] * ffn_nw[:, None]
            wdT[i * FD:(i + 1) * FD] = blk[2, e].T
            idx = p_idx if half == "p" else f_idx
            toks = np.unique(np.where((idx == e).any(1))[0])
            toks_il = 2 * toks + (0 if half == "p" else 1)
            col = np.full(NT * P, T2, np.int32)
            col[:toks_il.size] = toks_il
            gidx[:, i * NT:(i + 1) * NT] = col.reshape(NT, P).T
            ke = (idx == e)
            half_rows = t_even if half == "p" else t_odd
            for k in range(K):
                selE4[half_rows, k, i] = ke[:, k]
        selE4 = selE4.reshape(T2, K * 4)

        h = [2 * c0, 2 * c0 + 1]
        qcols = np.concatenate([np.arange(hh * HEAD, (hh + 1) * HEAD) for hh in h])
        wqkv = np.concatenate(
            [W_attn_n[:, qcols], W_attn_n[:, DIM + qcols], W_attn_n[:, 2 * DIM + qcols]], 1
        )
        bmask = np.zeros((T2, 1), np.float32)
        bmask[c0 * (T2 // NC):(c0 + 1) * (T2 // NC)] = 1.0

        per_core.append({
            "x_il": x_il,
            "cos128": cos128,
            "sin128s": sin128s,
            "masks": _bf(masks),
            "wqkv": _bf(wqkv),
            "wo": _bf(W_o),
            "wup": _bf(np.concatenate(
                [ffn_up_n[:, c0 * 256:(c0 + 1) * 256],
                 ffn_up_n[:, DS + c0 * 256: DS + (c0 + 1) * 256]], 1)),
            "wdn": _bf(ffn_dn[c0 * 256:(c0 + 1) * 256]),
            "keys": _bf(keys_cat),
            "sel": _f32(sel),
            "selE4": _f32(selE4),
            "valsadj": _f32(vals_adj),
            "bmask": bmask,
            "wg": _bf(wg),
            "wu": _bf(wu),
            "wdT": _bf(wdT),
            "gidx": gidx,
        })
    return per_core, meta


# --------------------------------------------------------------------------
# device program
# --------------------------------------------------------------------------

def _build(NT, debug=False):
    nc = bacc.Bacc("TRN2", target_bir_lowering=False, debug=False, num_devices=NC)

    x_il = nc.dram_tensor("x_il", [T2, DIM], F32, kind="ExternalInput")
    cos128 = nc.dram_tensor("cos128", [P, T2], F32, kind="ExternalInput")
    sin128s = nc.dram_tensor("sin128s", [P, T2], F32, kind="ExternalInput")
    masks = nc.dram_tensor("masks", [P, 4 * 512], BF, kind="ExternalInput")
    wqkv = nc.dram_tensor("wqkv", [DIM, 384], BF, kind="ExternalInput")
    wo = nc.dram_tensor("wo", [DIM, DIM], BF, kind="ExternalInput")
    wup = nc.dram_tensor("wup", [DIM, 512], BF, kind="ExternalInput")
    wdn = nc.dram_tensor("wdn", [256, DIM], BF, kind="ExternalInput")
    keys = nc.dram_tensor("keys", [DIM, 32], BF, kind="ExternalInput")
    sel = nc.dram_tensor("sel", [T2, K * 32], F32, kind="ExternalInput")
    selE4 = nc.dram_tensor("selE4", [T2, K * 4], F32, kind="ExternalInput")
    valsadj = nc.dram_tensor("valsadj", [T2, K], F32, kind="ExternalInput")
    bmask = nc.dram_tensor("bmask", [T2, 1], F32, kind="ExternalInput")
    wg = nc.dram_tensor("wg", [4 * DIM, FD], BF, kind="ExternalInput")
    wu = nc.dram_tensor("wu", [4 * DIM, FD], BF, kind="ExternalInput")
    wdT = nc.dram_tensor("wdT", [4 * FD, DIM], BF, kind="ExternalInput")
    gidx = nc.dram_tensor("gidx", [P, 4 * NT], I32, kind="ExternalInput")

    y_slice = nc.dram_tensor("y_slice", [T2 // NC, DIM], F32, kind="ExternalOutput")

    xffn_d = nc.dram_tensor("xffn_d", [TROWS, DIM], BF, kind="Internal")
    w_d = nc.dram_tensor("w_d", [TROWS, 4], F32, kind="Internal")
    yacc_d = nc.dram_tensor("yacc_d", [TROWS, DIM], F32, kind="Internal")
    xfi_d = nc.dram_tensor("xfi_d", [T2, DIM], F32, kind="Internal")
    oT_d = nc.dram_tensor("oT_d", [DIM, T2], BF, kind="Internal")

    dbg = {}
    if debug:
        for name, shp, dt in [
            ("dbg_obf", [P, T2], F32),
            ("dbg_xfi", [T2, DIM], F32),
            ("dbg_xffn", [T2, DIM], F32),
            ("dbg_w", [TROWS, 4], F32),
            ("dbg_yacc", [TROWS, DIM], F32),
            ("dbg_oall", [DIM, T2], F32),
        ]:
            dbg[name] = nc.dram_tensor(name, shp, dt, kind="ExternalOutput")

    with tile.TileContext(nc) as tc:
        with (
            tc.tile_pool(name="glob", bufs=1) as gpool,
            tc.tile_pool(name="dram", bufs=1, space="DRAM") as dpool,
        ):
            eps_t = gpool.tile([P, 1], F32)
            nc.vector.memset(eps_t[:], EPS)
            ones1x64 = gpool.tile([1, 64], BF)
            nc.vector.memset(ones1x64[:], 1.0)
            zeros_f = gpool.tile([P, DIM], F32)
            nc.vector.memset(zeros_f[:], 0.0)
            zeros_bf = gpool.tile([P, DIM], BF)
            nc.vector.memset(zeros_bf[:], 0.0)
            zero4 = gpool.tile([P, 4], F32)
            nc.vector.memset(zero4[:], 0.0)
            gidx_sb = gpool.tile([P, 4 * NT], I32)
            nc.sync.dma_start(gidx_sb[:], gidx[:])

            for t in range(TROWS // P):
                nc.sync.dma_start(yacc_d[t * P:(t + 1) * P, :], zeros_f[:])
            nc.sync.dma_start(xffn_d[T2:TROWS, :], zeros_bf[:])
            nc.sync.dma_start(w_d[T2:TROWS, :], zero4[:])

            # =================== ATTENTION SCOPE ===================
            with (
                tc.tile_pool(name="acst", bufs=1) as acst,
                tc.tile_pool(name="abig", bufs=1) as abig,
                tc.tile_pool(name="awork", bufs=2) as awork,
                tc.tile_pool(name="asmall", bufs=4) as asmall,
            ):
                cos_sb = acst.tile([P, T2], F32)
                nc.sync.dma_start(cos_sb[:], cos128[:])
                sin_sb = acst.tile([P, T2], F32)
                nc.sync.dma_start(sin_sb[:], sin128s[:])
                masks_sb = acst.tile([P, 4 * 512], BF)
                nc.sync.dma_start(masks_sb[:], masks[:])
                wqkv_sb = acst.tile([P, KC, 384], BF)
                nc.sync.dma_start(wqkv_sb[:], wqkv.ap().rearrange("(c p) m -> p c m", p=P))

                xT = abig.tile([P, KC, T2], BF)
                with tc.tile_pool(name="p1ps", bufs=2, space="PSUM") as p1ps:
                    for t in range(TC):
                        xc = awork.tile([P, DIM], F32, tag="xc")
                        nc.sync.dma_start(xc[:], x_il[t * P:(t + 1) * P, :])
                        sq = awork.tile([P, DIM], F32, tag="sq")
                        ssum = asmall.tile([P, 1], F32, tag="ssum")
                        nc.scalar.activation(sq[:], xc[:], AF.Square, accum_out=ssum[:])
                        rstd = asmall.tile([P, 1], F32, tag="rstd")
                        nc.scalar.activation(rstd[:], ssum[:], AF.Sqrt,
                                             scale=1.0 / DIM, bias=eps_t[:])
                        nc.vector.reciprocal(rstd[:], rstd[:])
                        xbf = awork.tile([P, DIM], BF, tag="xbf")
                        nc.vector.tensor_scalar_mul(xbf[:], xc[:], rstd[:])
                        nc.sync.dma_start_transpose(xT[:, :, t * P:(t + 1) * P], xbf[:])

                    # QKV (transposed layouts)
                    qTf = abig.tile([P, T2], F32)
                    kTf = abig.tile([P, T2], F32)
                    vTf = abig.tile([P, T2], F32)
                    for tgt, dst in ((0, qTf), (1, kTf), (2, vTf)):
                        for nb in range(NB):
                            ps = p1ps.tile([P, 512], F32, tag="mm")
                            for kc in range(KC):
                                nc.tensor.matmul(
                                    ps[:],
                                    wqkv_sb[:, kc, tgt * P:(tgt + 1) * P],
                                    xT[:, kc, nb * 512:(nb + 1) * 512],
                                    start=(kc == 0), stop=(kc == KC - 1),
                                )
                            nc.scalar.activation(dst[:, nb * 512:(nb + 1) * 512],
                                                 ps[:], AF.Copy)

                # RoPE + casts
                qbf = abig.tile([P, T2], BF)
                kbf = abig.tile([P, T2], BF)
                with tc.tile_pool(name="rwork", bufs=1) as rwork:
                    for src, dst in ((qTf, qbf), (kTf, kbf)):
                        rot = rwork.tile([P, T2], F32, tag="rot")
                        for g in range(4):
                            lo = g * 32
                            slo = lo + 32 if g % 2 == 0 else lo - 32
                            nc.vector.tensor_copy(rot[lo:lo + 32, :], src[slo:slo + 32, :])
                        tmp = rwork.tile([P, T2], F32, tag="ropetmp")
                        nc.vector.tensor_mul(tmp[:], rot[:], sin_sb[:])
                        nc.vector.tensor_mul(rot[:], src[:], cos_sb[:])
                        nc.vector.tensor_add(dst[:], rot[:], tmp[:])

                    vbf = rwork.tile([P, T2], BF, tag="vbf")
                    nc.vector.tensor_copy(vbf[:], vTf[:])
                    vnat = abig.tile([P, TC, P], BF)
                    nc.sync.dma_start_transpose(vnat[:], vbf[:])
                vaug = abig.tile([P, TC, 2, 65], BF)
                nc.vector.tensor_copy(vaug[:, :, 0, 0:64], vnat[:, :, 0:64])
                nc.vector.tensor_copy(vaug[:, :, 1, 0:64], vnat[:, :, 64:128])
                nc.vector.memset(vaug[:, :, :, 64], 1.0)

                # causal attention
                obf = abig.tile([P, T2], BF)
                with (
                    tc.tile_pool(name="aps", bufs=2, space="PSUM") as aps,
                    tc.tile_pool(name="ops_p", bufs=2, space="PSUM") as ops_p,
                ):
                    for h in range(2):
                        hs = h * 64
                        for qb in range(NB):
                            ops_ = ops_p.tile([65, 512], F32, tag="ops")
                            nkb = 4 * qb + 4
                            for kb in range(nkb):
                                sps = aps.tile([P, 512], F32, tag="sps")
                                nc.tensor.matmul(
                                    sps[:],
                                    kbf[hs:hs + 64, kb * P:(kb + 1) * P],
                                    qbf[hs:hs + 64, qb * 512:(qb + 1) * 512],
                                    start=True, stop=True,
                                )
                                pbf = awork.tile([P, 512], BF, tag="pbf")
                                nc.scalar.activation(pbf[:], sps[:], AF.Exp, scale=0.125)
                                if kb >= 4 * qb:
                                    j = kb - 4 * qb
                                    nc.vector.tensor_mul(
                                        pbf[:], pbf[:], masks_sb[:, j * 512:(j + 1) * 512])
                                nc.tensor.matmul(
                                    ops_[:], vaug[:, kb, h, :], pbf[:],
                                    start=(kb == 0), stop=(kb == nkb - 1),
                                )
                            linv = asmall.tile([1, 512], F32, tag="linv")
                            nc.vector.reciprocal(linv[:], ops_[64:65, :])
                            linb = asmall.tile([1, 512], BF, tag="linb")
                            nc.vector.tensor_copy(linb[:], linv[:])
                            bps = aps.tile([64, 512], F32, tag="bps")
                            nc.tensor.matmul(bps[:], ones1x64[:], linb[:],
                                             start=True, stop=True)
                            bsb = awork.tile([64, 512], F32, tag="bsb")
                            nc.vector.tensor_copy(bsb[:], bps[:])
                            nc.vector.tensor_mul(
                                obf[hs:hs + 64, qb * 512:(qb + 1) * 512],
                                ops_[0:64, :], bsb[:])
                if debug:
                    ot = awork.tile([P, T2], F32, tag="odbg")
                    nc.vector.tensor_copy(ot[:], obf[:])
                    nc.sync.dma_start(dbg["dbg_obf"][:], ot[:])

                # AllGather o^T
                ag_in = dpool.tile([P, T2], BF)
                nc.gpsimd.dma_start(ag_in[:], obf[:])
                nc.gpsimd.collective_compute(
                    "AllGather", OP.bypass,
                    replica_groups=[list(range(NC))],
                    ins=[ag_in.opt()], outs=[oT_d.ap()],
                )

            # =================== FFN SCOPE ===================
            with (
                tc.tile_pool(name="fcst", bufs=1) as fcst,
                tc.tile_pool(name="fbig", bufs=1) as fbig,
                tc.tile_pool(name="fwork", bufs=2) as fwork,
                tc.tile_pool(name="fsmall", bufs=4) as fsmall,
            ):
                wo_sb = fcst.tile([P, KC, DIM], BF)
                nc.sync.dma_start(wo_sb[:], wo.ap().rearrange("(c p) m -> p c m", p=P))
                wup_sb = fcst.tile([P, KC, 512], BF)
                nc.sync.dma_start(wup_sb[:], wup.ap().rearrange("(c p) m -> p c m", p=P))
                wdn_sb = fcst.tile([P, 2, DIM], BF)
                nc.sync.dma_start(wdn_sb[:], wdn.ap().rearrange("(c p) m -> p c m", p=P))
                keys_sb = fcst.tile([P, KC, 32], BF)
                nc.sync.dma_start(keys_sb[:], keys.ap().rearrange("(c p) m -> p c m", p=P))

                xfT = fbig.tile([P, KC, T2], BF)

                # o-proj + residual + rmsnorm
                with (
                    tc.tile_pool(name="p6", bufs=1) as p6,
                    tc.tile_pool(name="p6w", bufs=2) as p6w,
                    tc.tile_pool(name="p6ps", bufs=4, space="PSUM") as p6ps,
                ):
                    oT = p6.tile([P, KC, T2], BF)
                    nc.sync.dma_start(oT[:], oT_d.ap().rearrange("(c p) t -> p c t", p=P))
                    if debug:
                        for kc2 in range(KC):
                            ot2 = p6w.tile([P, T2], F32, tag="odbg2")
                            nc.vector.tensor_copy(ot2[:], oT[:, kc2, :])
                            nc.sync.dma_start(dbg["dbg_oall"][kc2 * P:(kc2 + 1) * P, :], ot2[:])
                    for t in range(TC):
                        xfi = p6w.tile([P, DIM], F32, tag="xfi")
                        xc = p6w.tile([P, DIM], F32, tag="xc")
                        nc.sync.dma_start(xc[:], x_il[t * P:(t + 1) * P, :])
                        for nh in range(2):
                            ps = p6ps.tile([P, 512], F32, tag="mm")
                            for kc in range(KC):
                                nc.tensor.matmul(
                                    ps[:],
                                    oT[:, kc, t * P:(t + 1) * P],
                                    wo_sb[:, kc, nh * 512:(nh + 1) * 512],
                                    start=(kc == 0), stop=(kc == KC - 1),
                                )
                            nc.vector.tensor_add(
                                xfi[:, nh * 512:(nh + 1) * 512], ps[:],
                                xc[:, nh * 512:(nh + 1) * 512])
                        nc.sync.dma_start(xfi_d[t * P:(t + 1) * P, :], xfi[:])
                        sq = p6w.tile([P, DIM], F32, tag="sq")
                        ssum = fsmall.tile([P, 1], F32, tag="ssum")
                        nc.scalar.activation(sq[:], xfi[:], AF.Square, accum_out=ssum[:])
                        rstd = fsmall.tile([P, 1], F32, tag="rstd")
                        nc.scalar.activation(rstd[:], ssum[:], AF.Sqrt,
                                             scale=1.0 / DIM, bias=eps_t[:])
                        nc.vector.reciprocal(rstd[:], rstd[:])
                        xfbf = p6w.tile([P, DIM], BF, tag="xfbf")
                        nc.vector.tensor_scalar_mul(xfbf[:], xfi[:], rstd[:])
                        nc.sync.dma_start(xffn_d[t * P:(t + 1) * P, :], xfbf[:])
                        nc.sync.dma_start_transpose(xfT[:, :, t * P:(t + 1) * P], xfbf[:])

                # router
                with (
                    tc.tile_pool(name="p7c", bufs=1) as p7c,
                    tc.tile_pool(name="p7ps", bufs=2, space="PSUM") as p7ps,
                ):
                    sel_sb = p7c.tile([P, TC, K * 32], F32)
                    nc.sync.dma_start(sel_sb[:], sel.ap().rearrange("(t p) m -> p t m", p=P))
                    selE_sb = p7c.tile([P, TC, K * 4], F32)
                    nc.sync.dma_start(selE_sb[:], selE4.ap().rearrange("(t p) m -> p t m", p=P))
                    va_sb = p7c.tile([P, TC, K], F32)
                    nc.sync.dma_start(va_sb[:], valsadj.ap().rearrange("(t p) m -> p t m", p=P))

                    for t in range(TC):
                        lps = p7ps.tile([P, 32], F32, tag="lps")
                        for kc in range(KC):
                            nc.tensor.matmul(
                                lps[:], xfT[:, kc, t * P:(t + 1) * P], keys_sb[:, kc, :],
                                start=(kc == 0), stop=(kc == KC - 1),
                            )
                        lg = fsmall.tile([P, 32], F32, tag="lg")
                        nc.vector.tensor_copy(lg[:], lps[:])
                        gath = fsmall.tile([P, K], F32, tag="gath")
                        prod = fsmall.tile([P, 32], F32, tag="prod")
                        for k in range(K):
                            nc.vector.tensor_mul(prod[:], lg[:],
                                                 sel_sb[:, t, k * 32:(k + 1) * 32])
                            nc.vector.reduce_sum(gath[:, k:k + 1], prod[:],
                                                 axis=mybir.AxisListType.X)
                        nc.vector.tensor_add(gath[:], gath[:], va_sb[:, t, :])
                        nc.scalar.activation(gath[:], gath[:], AF.Sigmoid)
                        ssum2 = fsmall.tile([P, 1], F32, tag="ssum2")
                        nc.vector.reduce_sum(ssum2[:], gath[:], axis=mybir.AxisListType.X)
                        nc.vector.reciprocal(ssum2[:], ssum2[:])
                        nc.vector.tensor_scalar_mul(gath[:], gath[:], ssum2[:])
                        w4 = fsmall.tile([P, 4], F32, tag="w4")
                        nc.vector.tensor_scalar_mul(w4[:], selE_sb[:, t, 0:4], gath[:, 0:1])
                        for k in range(1, K):
                            nc.vector.scalar_tensor_tensor(
                                out=w4[:], in0=selE_sb[:, t, k * 4:(k + 1) * 4],
                                scalar=gath[:, k:k + 1], in1=w4[:],
                                op0=OP.mult, op1=OP.add,
                            )
                        nc.sync.dma_start(w_d[t * P:(t + 1) * P, :], w4[:])
                if debug:
                    nc.sync.dma_start(dbg["dbg_xfi"][:], xfi_d[:])
                    nc.sync.dma_start(dbg["dbg_w"][:], w_d[:])
                    with tc.tile_pool(name="dbgp", bufs=2) as dbgp:
                        for t in range(TC):
                            xfb2 = dbgp.tile([P, DIM], BF, tag="xfb2")
                            nc.sync.dma_start(xfb2[:], xffn_d[t * P:(t + 1) * P, :])
                            dbx = dbgp.tile([P, DIM], F32, tag="dbx")
                            nc.vector.tensor_copy(dbx[:], xfb2[:])
                            nc.sync.dma_start(dbg["dbg_xffn"][t * P:(t + 1) * P, :], dbx[:])

                # experts
                with (
                    tc.tile_pool(name="p8w", bufs=2) as p8w,
                    tc.tile_pool(name="p8ps", bufs=2, space="PSUM") as p8ps,
                ):
                    for i in range(4):
                        wg_sb = p8w.tile([P, KC, FD], BF, tag="wg")
                        nc.sync.dma_start(
                            wg_sb[:],
                            wg[i * DIM:(i + 1) * DIM, :].rearrange("(c p) m -> p c m", p=P))
                        wu_sb = p8w.tile([P, KC, FD], BF, tag="wu")
                        nc.sync.dma_start(
                            wu_sb[:],
                            wu[i * DIM:(i + 1) * DIM, :].rearrange("(c p) m -> p c m", p=P))
                        wdT_sb = p8w.tile([P, 4, DIM], BF, tag="wdT")
                        nc.sync.dma_start(
                            wdT_sb[:],
                            wdT[i * FD:(i + 1) * FD, :].rearrange("(c p) m -> p c m", p=P))
                        for r0 in range(NT):
                            col = i * NT + r0
                            gx = p8w.tile([P, DIM], BF, tag="gx")
                            nc.gpsimd.indirect_dma_start(
                                out=gx[:], out_offset=None, in_=xffn_d[:],
                                in_offset=bass.IndirectOffsetOnAxis(
                                    ap=gidx_sb[:, col:col + 1], axis=0),
                            )
                            gxT = p8w.tile([P, KC, P], BF, tag="gxT")
                            nc.sync.dma_start_transpose(gxT[:], gx[:])
                            gw = fsmall.tile([P, 4], F32, tag="gw")
                            nc.gpsimd.indirect_dma_start(
                                out=gw[:], out_offset=None, in_=w_d[:],
                                in_offset=bass.IndirectOffsetOnAxis(
                                    ap=gidx_sb[:, col:col + 1], axis=0),
                            )
                            wv = fsmall.tile([P, 1], F32, tag="wv")
                            nc.vector.tensor_copy(wv[:], gw[:, i:i + 1])

                            gps = p8ps.tile([P, 512], F32, tag="gps")
                            ups = p8ps.tile([P, 512], F32, tag="ups")
                            for fc in range(4):
                                for kc in range(KC):
                                    nc.tensor.matmul(
                                        gps[:, fc * P:(fc + 1) * P],
                                        wg_sb[:, kc, fc * P:(fc + 1) * P],
                                        gxT[:, kc, :],
                                        start=(kc == 0), stop=(kc == KC - 1),
                                    )
                                for kc in range(KC):
                                    nc.tensor.matmul(
                                        ups[:, fc * P:(fc + 1) * P],
                                        wu_sb[:, kc, fc * P:(fc + 1) * P],
                                        gxT[:, kc, :],
                                        start=(kc == 0), stop=(kc == KC - 1),
                                    )
                            sg = p8w.tile([P, 512], BF, tag="sg")
                            nc.scalar.activation(sg[:], gps[:], AF.Silu)
                            hT = p8w.tile([P, 512], BF, tag="hT")
                            nc.vector.tensor_mul(hT[:], sg[:], ups[:])
                            ysb = p8w.tile([P, DIM], F32, tag="ysb")
                            for nh in range(2):
                                dps = p8ps.tile([P, 512], F32, tag="dps")
                                for fc in range(4):
                                    nc.tensor.matmul(
                                        dps[:],
                                        hT[:, fc * P:(fc + 1) * P],
                                        wdT_sb[:, fc, nh * 512:(nh + 1) * 512],
                                        start=(fc == 0), stop=(fc == 3),
                                    )
                                nc.vector.tensor_scalar_mul(
                                    ysb[:, nh * 512:(nh + 1) * 512], dps[:], wv[:])
                            nc.gpsimd.indirect_dma_start(
                                out=yacc_d[:],
                                out_offset=bass.IndirectOffsetOnAxis(
                                    ap=gidx_sb[:, col:col + 1], axis=0),
                                in_=ysb[:], in_offset=None,
                                compute_op=OP.add,
                            )
                if debug:
                    nc.sync.dma_start(dbg["dbg_yacc"][:], yacc_d[:])

                # shared expert up
                hsT = fbig.tile([P, 2, T2], BF)
                with (
                    tc.tile_pool(name="p9w", bufs=2) as p9w,
                    tc.tile_pool(name="p9ps", bufs=2, space="PSUM") as p9ps,
                ):
                    for m in range(2):
                        for nb in range(NB):
                            x1ps = p9ps.tile([P, 512], F32, tag="x1ps")
                            for kc in range(KC):
                                nc.tensor.matmul(
                                    x1ps[:], wup_sb[:, kc, m * P:(m + 1) * P],
                                    xfT[:, kc, nb * 512:(nb + 1) * 512],
                                    start=(kc == 0), stop=(kc == KC - 1),
                                )
                            sg1 = p9w.tile([P, 512], BF, tag="sg1")
                            nc.scalar.activation(sg1[:], x1ps[:], AF.Silu)
                            x2ps = p9ps.tile([P, 512], F32, tag="x2ps")
                            for kc in range(KC):
                                nc.tensor.matmul(
                                    x2ps[:], wup_sb[:, kc, (m + 2) * P:(m + 3) * P],
                                    xfT[:, kc, nb * 512:(nb + 1) * 512],
                                    start=(kc == 0), stop=(kc == KC - 1),
                                )
                            nc.vector.tensor_mul(hsT[:, m, nb * 512:(nb + 1) * 512],
                                                 sg1[:], x2ps[:])

                # shared down + combine + ReduceScatter
                rs_in = dpool.tile([T2, DIM], F32)
                rs_out = dpool.tile([T2 // NC, DIM], F32)
                with (
                    tc.tile_pool(name="p10c", bufs=1) as p10c,
                    tc.tile_pool(name="p10w", bufs=2) as p10w,
                    tc.tile_pool(name="p10ps", bufs=2, space="PSUM") as p10ps,
                ):
                    bm_sb = p10c.tile([P, TC, 1], F32)
                    nc.sync.dma_start(bm_sb[:], bmask.ap().rearrange("(t p) m -> p t m", p=P))
                    for t in range(TC):
                        yacc_sb = p10w.tile([P, DIM], F32, tag="yacc_sb")
                        nc.sync.dma_start(yacc_sb[:], yacc_d[t * P:(t + 1) * P, :])
                        xfi2 = p10w.tile([P, DIM], F32, tag="xfi2")
                        nc.sync.dma_start(xfi2[:], xfi_d[t * P:(t + 1) * P, :])
                        ycmb = p10w.tile([P, DIM], F32, tag="ycmb")
                        for nh in range(2):
                            sdps = p10ps.tile([P, 512], F32, tag="sdps")
                            for fc in range(2):
                                nc.tensor.matmul(
                                    sdps[:],
                                    hsT[:, fc, t * P:(t + 1) * P],
                                    wdn_sb[:, fc, nh * 512:(nh + 1) * 512],
                                    start=(fc == 0), stop=(fc == 1),
                                )
                            nc.vector.scalar_tensor_tensor(
                                out=ycmb[:, nh * 512:(nh + 1) * 512],
                                in0=xfi2[:, nh * 512:(nh + 1) * 512],
                                scalar=bm_sb[:, t, :], in1=sdps[:],
                                op0=OP.mult, op1=OP.add,
                            )
                        nc.vector.tensor_add(ycmb[:], ycmb[:], yacc_sb[:])
                        nc.sync.dma_start(rs_in[t * P:(t + 1) * P, :], ycmb[:])
                nc.gpsimd.collective_compute(
                    "ReduceScatter", OP.add,
                    replica_groups=[list(range(NC))],
                    ins=[rs_in.opt()], outs=[rs_out.opt()],
                )
                nc.sync.dma_start(y_slice[:], rs_out[:])

    nc.compile()
    return nc


_CACHE = {}


def _get_nc(NT, debug=False):
    key = (NT, debug)
    if key not in _CACHE:
        _CACHE[key] = _build(NT, debug)
    return _CACHE[key]


def run(inputs, debug=False, trace=False):
    per_core, meta = _prepare(inputs)
    nc = _get_nc(meta["NT"], debug)
    res = run_bass_kernel_spmd(nc, per_core, core_ids=list(range(NC)), trace=trace)
    y_il = np.concatenate([res.results[c]["y_slice"] for c in range(NC)], 0)
    y = np.stack([y_il[0::2], y_il[1::2]], 0).astype(np.float32)
    return y, res


def kernel(**inputs):
    y, _ = run(inputs)
    return y
